# Initial kernel scaffold
#
"""MoE top-1 routing (fairseq Top1Gate style) on 8 trn2 NeuronCores.

Strategy:
  - Host (numpy, float64): gate logits, softmax, argmax, capacity cumsum,
    l_aux.  This is the shard-construction step: it produces, for each
    expert, the list of kept tokens and their capacity slots.
  - Shard: expert-parallel x token-parallel.  Core 2e+j handles expert e's
    capacity slots [j*1024, (j+1)*1024).  Each core receives its tokens
    already gathered AND transposed ([M, 1024] bf16) plus its expert's
    fc1/fc2 weights (bf16) and fc1 bias.
  - Device (Bass/Tile, bf16 matmuls, fp32 accum):
      hT = gelu_tanh(W1.T-contracted xt + b1)   # [H, tok] layout
      out = hT.T-contracted W2                  # [tok, O] fp32
  - Host combine: out_full[token] = gate * (core_out[slot] + fc2_b[e]).

Self-contained: shapes hardcoded for B=8,T=1024,M=1024,H=4096,O=1024,E=4.
"""

import os
import numpy as np
import ml_dtypes

B, T, M, H, O, E = 8, 1024, 1024, 4096, 1024, 4
S = B * T            # 8192 tokens
C = 2048             # capacity = ceil(S/E) * 1.0
NCORES = 8
TOK = C // 2         # tokens per core
P = 128

LAST_RESULTS = None  # test.py introspection (BassKernelResults)
_NC_CACHE = None

BF16 = ml_dtypes.bfloat16


# --------------------------------------------------------------------------
# Host routing (the gate + shard construction)
# --------------------------------------------------------------------------

def _route(features, gate_w):
    """Returns (idx, keep, gate_val, l_aux, per-expert token lists)."""
    lg = features.astype(np.float64) @ gate_w.astype(np.float64)      # [S, E]
    idx = np.argmax(lg, axis=1)
    z = lg - lg.max(axis=1, keepdims=True)
    ez = np.exp(z)
    gates = ez / ez.sum(axis=1, keepdims=True)                        # [S, E]

    mask1 = np.zeros((S, E), np.float64)
    mask1[np.arange(S), idx] = 1.0
    me = gates.mean(axis=0)
    ce = mask1.mean(axis=0)
    l_aux = np.float32((me * ce).sum() * E)

    # position of each token within its expert's queue (token order)
    locations = np.cumsum(mask1, axis=0) - 1.0
    loc = locations[np.arange(S), idx].astype(np.int64)
    keep = loc < C
    gate_val = (gates[np.arange(S), idx] * keep).astype(np.float32)

    toks_per_e = [np.nonzero((idx == e) & keep)[0] for e in range(E)]
    return idx, keep, gate_val, l_aux, toks_per_e


# --------------------------------------------------------------------------
# Device kernel (Bass/Tile): per-core expert MLP chunk
# --------------------------------------------------------------------------

def _build_nc():
    import concourse.bacc as bacc
    import concourse.mybir as mybir
    from concourse import tile

    F32 = mybir.dt.float32
    DBF16 = mybir.dt.bfloat16
    GELU = mybir.ActivationFunctionType.Gelu_apprx_tanh

    KM = M // P            # 8  k-chunks for GEMM1
    NH = H // P            # 32 h-tiles
    HC = 4                 # w1 stream chunks
    NH_C = NH // HC        # 8  h-tiles per chunk

    nc = bacc.Bacc(None, target_bir_lowering=False)
    xt = nc.declare_dram_parameter("xt", [M, TOK], DBF16, isOutput=False)
    w1 = nc.declare_dram_parameter("w1", [M, H], DBF16, isOutput=False)
    b1 = nc.declare_dram_parameter("b1", [P, NH], F32, isOutput=False)
    w2 = nc.declare_dram_parameter("w2", [H, O], DBF16, isOutput=False)
    out = nc.declare_dram_parameter("out", [TOK, O], F32, isOutput=True)

    xt_r = xt[:].rearrange("(k p) t -> p k t", p=P)     # [128, 8, 1024]
    w1_r = w1[:].rearrange("(k p) h -> p k h", p=P)     # [128, 8, 4096]
    w2_r = w2[:].rearrange("(k p) o -> p k o", p=P)     # [128, 32, 1024]

    with tile.TileContext(nc) as tc:
        with (
            tc.tile_pool(name="xt_pool", bufs=1) as xt_pool,
            tc.tile_pool(name="b1_pool", bufs=1) as b1_pool,
            tc.tile_pool(name="hT_pool", bufs=1) as hT_pool,
            tc.tile_pool(name="w1_pool", bufs=2) as w1_pool,
            tc.tile_pool(name="w2_pool", bufs=1) as w2_pool,
            tc.tile_pool(name="out_pool", bufs=4) as out_pool,
        ):
            xt_sb = xt_pool.tile([P, KM, TOK], DBF16)
            nc.sync.dma_start(out=xt_sb[:], in_=xt_r)
            b1_sb = b1_pool.tile([P, NH], F32)
            nc.sync.dma_start(out=b1_sb[:], in_=b1[:])
            # w2 resident for all of phase B; DMA overlaps phase A compute
            w2_sb = w2_pool.tile([P, NH, O], DBF16)
            nc.sync.dma_start(out=w2_sb[:], in_=w2_r)

            hT = hT_pool.tile([P, NH, TOK], DBF16)      # [H-part, h-tile, tok]

            # ---- phase A: hT = gelu(W1.T @ X.T + b1), H-major layout ----
            with tc.tile_pool(name="psA", bufs=4, space="PSUM") as psA:
                for hc in range(HC):
                    w1c = w1_pool.tile([P, KM, NH_C * P], DBF16, tag="w1c")
                    nc.sync.dma_start(
                        out=w1c[:],
                        in_=w1_r[:, :, hc * NH_C * P:(hc + 1) * NH_C * P],
                    )
                    for h in range(NH_C):
                        h_abs = hc * NH_C + h
                        for t in range(TOK // 512):
                            ps = psA.tile([P, 512], F32, tag="ps")
                            for k in range(KM):
                                nc.tensor.matmul(
                                    ps[:],
                                    lhsT=w1c[:, k, h * P:(h + 1) * P],
                                    rhs=xt_sb[:, k, t * 512:(t + 1) * 512],
                                    start=(k == 0),
                                    stop=(k == KM - 1),
                                )
                            nc.scalar.activation(
                                hT[:, h_abs, t * 512:(t + 1) * 512],
                                ps[:],
                                GELU,
                                bias=b1_sb[:, h_abs:h_abs + 1],
                            )

            # ---- phase B: out = hT.T @ W2, 512 tokens at a time ----
            with tc.tile_pool(name="psB", bufs=1, space="PSUM") as psB_pool:
                psB = psB_pool.tile([P, 8 * 512], F32)   # all 8 banks
                for half in range(2):
                    for hk in range(NH):
                        for t in range(4):
                            tt = half * 4 + t
                            for oc in range(2):
                                r = t * 2 + oc
                                nc.tensor.matmul(
                                    psB[:, r * 512:(r + 1) * 512],
                                    lhsT=hT[:, hk, tt * P:(tt + 1) * P],
                                    rhs=w2_sb[:, hk, oc * 512:(oc + 1) * 512],
                                    start=(hk == 0),
                                    stop=(hk == NH - 1),
                                )
                    for t in range(4):
                        tt = half * 4 + t
                        for oc in range(2):
                            r = t * 2 + oc
                            ot = out_pool.tile([P, 512], F32, tag="ot")
                            nc.vector.tensor_copy(ot[:], psB[:, r * 512:(r + 1) * 512])
                            nc.sync.dma_start(
                                out=out[tt * P:(tt + 1) * P, oc * 512:(oc + 1) * 512],
                                in_=ot[:],
                            )

    nc.compile()
    return nc


def _run_device(in_maps):
    global LAST_RESULTS, _NC_CACHE
    from concourse.bass_utils import run_bass_kernel_spmd

    if _NC_CACHE is None:
        _NC_CACHE = _build_nc()
    res = run_bass_kernel_spmd(_NC_CACHE, in_maps, core_ids=list(range(NCORES)))
    LAST_RESULTS = res
    return [r["out"] for r in res.results]


def _run_numpy(in_maps):
    """Host fallback mirroring the device math (for routing validation)."""
    outs = []
    for m in in_maps:
        x = m["xt"].astype(np.float32).T           # [tok, M]
        w1 = m["w1"].astype(np.float32)
        w2 = m["w2"].astype(np.float32)
        b1 = m["b1"].T.reshape(-1)                 # [H]
        h = x @ w1 + b1
        g = 0.5 * h * (1.0 + np.tanh(0.7978845608028654 * (h + 0.044715 * h ** 3)))
        outs.append((g @ w2).astype(np.float32))
    return outs


# --------------------------------------------------------------------------
# Entry point
# --------------------------------------------------------------------------

def kernel(hidden_states, gate_w, fc1_w, fc1_b, fc2_w, fc2_b):
    hidden_states = np.asarray(hidden_states)
    features = hidden_states.reshape(S, M)
    gate_w = np.asarray(gate_w)
    fc1_w, fc1_b = np.asarray(fc1_w), np.asarray(fc1_b)
    fc2_w, fc2_b = np.asarray(fc2_w), np.asarray(fc2_b)

    idx, keep, gate_val, l_aux, toks_per_e = _route(features, gate_w)

    # ---- shard: gather+transpose tokens per core, cast weights to bf16 ----
    in_maps = []
    for core in range(NCORES):
        e, half = core // 2, core % 2
        toks = toks_per_e[e]
        lo, hi = half * TOK, min((half + 1) * TOK, len(toks))
        xt = np.zeros((M, TOK), BF16)
        if hi > lo:
            xt[:, :hi - lo] = features[toks[lo:hi]].astype(BF16).T
        in_maps.append({
            "xt": xt,
            "w1": fc1_w[e].astype(BF16),
            "b1": np.ascontiguousarray(fc1_b[e].reshape(H // P, P).T.astype(np.float32)),
            "w2": fc2_w[e].astype(BF16),
        })

    if os.environ.get("MOE_NUMPY_MLP"):
        core_outs = _run_numpy(in_maps)
    else:
        core_outs = _run_device(in_maps)

    # ---- combine: scatter expert outputs back to token order ----
    out_full = np.zeros((S, O), np.float32)
    for e in range(E):
        toks = toks_per_e[e]
        n = len(toks)
        eo = np.concatenate([core_outs[2 * e], core_outs[2 * e + 1]], axis=0)[:n]
        eo = eo + fc2_b[e][None, :].astype(np.float32)
        out_full[toks] = gate_val[toks, None] * eo

    return out_full.reshape(B, T, O), l_aux


# revision 2
# speedup vs baseline: 17.5349x; 17.5349x over previous
"""MoE top-1 routing (fairseq Top1Gate style) on 8 trn2 NeuronCores.

Strategy:
  - Host (numpy, float64): gate logits, softmax, argmax, capacity cumsum,
    l_aux.  This is the shard-construction step: it produces, for each
    expert, the list of kept tokens and their capacity slots.
  - Shard: expert-parallel x token-parallel.  Core 2e+j handles expert e's
    capacity slots [j*1024, (j+1)*1024).  Each core receives its tokens
    already gathered AND transposed ([M, 1024] bf16) plus its expert's
    fc1/fc2 weights (bf16) and fc1 bias.
  - Device (Bass/Tile, bf16 matmuls, fp32 accum):
      hT = gelu_tanh(W1.T-contracted xt + b1)   # [H, tok] layout
      out = hT.T-contracted W2                  # [tok, O] fp32
  - Host combine: out_full[token] = gate * (core_out[slot] + fc2_b[e]).

Self-contained: shapes hardcoded for B=8,T=1024,M=1024,H=4096,O=1024,E=4.
"""

import os
import numpy as np
import ml_dtypes

B, T, M, H, O, E = 8, 1024, 1024, 4096, 1024, 4
S = B * T            # 8192 tokens
C = 2048             # capacity = ceil(S/E) * 1.0
NCORES = 8
TOK = C // 2         # tokens per core
P = 128

LAST_RESULTS = None  # test.py introspection (BassKernelResults)
_NC_CACHE = None

BF16 = ml_dtypes.bfloat16


# --------------------------------------------------------------------------
# Host routing (the gate + shard construction)
# --------------------------------------------------------------------------

def _route(features, gate_w):
    """Returns (idx, keep, gate_val, l_aux, per-expert token lists)."""
    lg = features.astype(np.float64) @ gate_w.astype(np.float64)      # [S, E]
    idx = np.argmax(lg, axis=1)
    z = lg - lg.max(axis=1, keepdims=True)
    ez = np.exp(z)
    gates = ez / ez.sum(axis=1, keepdims=True)                        # [S, E]

    mask1 = np.zeros((S, E), np.float64)
    mask1[np.arange(S), idx] = 1.0
    me = gates.mean(axis=0)
    ce = mask1.mean(axis=0)
    l_aux = np.float32((me * ce).sum() * E)

    # position of each token within its expert's queue (token order)
    locations = np.cumsum(mask1, axis=0) - 1.0
    loc = locations[np.arange(S), idx].astype(np.int64)
    keep = loc < C
    gate_val = (gates[np.arange(S), idx] * keep).astype(np.float32)

    toks_per_e = [np.nonzero((idx == e) & keep)[0] for e in range(E)]
    return idx, keep, gate_val, l_aux, toks_per_e


# --------------------------------------------------------------------------
# Device kernel (Bass/Tile): per-core expert MLP chunk
# --------------------------------------------------------------------------

def _build_nc(reps=1):
    import concourse.bacc as bacc
    import concourse.mybir as mybir
    from concourse import tile

    F32 = mybir.dt.float32
    DBF16 = mybir.dt.bfloat16
    GELU = mybir.ActivationFunctionType.Gelu_apprx_tanh

    KM = M // P            # 8  k-chunks for GEMM1
    NH = H // P            # 32 h-tiles
    HC = 4                 # w1 stream chunks
    NH_C = NH // HC        # 8  h-tiles per chunk

    nc = bacc.Bacc(None, target_bir_lowering=False)
    xt = nc.declare_dram_parameter("xt", [M, TOK], DBF16, isOutput=False)
    w1 = nc.declare_dram_parameter("w1", [M, H], DBF16, isOutput=False)
    b1 = nc.declare_dram_parameter("b1", [P, NH], F32, isOutput=False)
    w2 = nc.declare_dram_parameter("w2", [H, O], DBF16, isOutput=False)
    out = nc.declare_dram_parameter("out", [TOK, O], F32, isOutput=True)

    xt_r = xt[:].rearrange("(k p) t -> p k t", p=P)     # [128, 8, 1024]
    w1_r = w1[:].rearrange("(k p) h -> p k h", p=P)     # [128, 8, 4096]
    w2_r = w2[:].rearrange("(k p) o -> p k o", p=P)     # [128, 32, 1024]

    with tile.TileContext(nc) as tc:
        for rep in range(reps):
            _emit_body(nc, tc, mybir, xt_r, w1_r, b1, w2_r, out,
                       F32, DBF16, GELU, KM, NH, HC, NH_C, rep)

    nc.compile()
    return nc


def _emit_body(nc, tc, mybir, xt_r, w1_r, b1, w2_r, out,
               F32, DBF16, GELU, KM, NH, HC, NH_C, rep):
    sfx = f"_{rep}"
    with (
        tc.tile_pool(name="xt_pool" + sfx, bufs=1) as xt_pool,
        tc.tile_pool(name="b1_pool" + sfx, bufs=1) as b1_pool,
        tc.tile_pool(name="hT_pool" + sfx, bufs=1) as hT_pool,
        tc.tile_pool(name="w1_pool" + sfx, bufs=2) as w1_pool,
        tc.tile_pool(name="w2_pool" + sfx, bufs=1) as w2_pool,
        tc.tile_pool(name="out_pool" + sfx, bufs=4) as out_pool,
    ):
        xt_sb = xt_pool.tile([P, KM, TOK], DBF16)
        nc.sync.dma_start(out=xt_sb[:], in_=xt_r)
        b1_sb = b1_pool.tile([P, NH], F32)
        nc.sync.dma_start(out=b1_sb[:], in_=b1[:])
        # w2 resident for all of phase B; DMA overlaps phase A compute
        w2_sb = w2_pool.tile([P, NH, O], DBF16)
        nc.sync.dma_start(out=w2_sb[:], in_=w2_r)

        hT = hT_pool.tile([P, NH, TOK], DBF16)      # [H-part, h-tile, tok]

        # ---- phase A: hT = gelu(W1.T @ X.T + b1), H-major layout ----
        with tc.tile_pool(name="psA" + sfx, bufs=4, space="PSUM") as psA:
            for hc in range(HC):
                w1c = w1_pool.tile([P, KM, NH_C * P], DBF16, tag="w1c")
                nc.sync.dma_start(
                    out=w1c[:],
                    in_=w1_r[:, :, hc * NH_C * P:(hc + 1) * NH_C * P],
                )
                for h in range(NH_C):
                    h_abs = hc * NH_C + h
                    for t in range(TOK // 512):
                        ps = psA.tile([P, 512], F32, tag="ps")
                        for k in range(KM):
                            nc.tensor.matmul(
                                ps[:],
                                lhsT=w1c[:, k, h * P:(h + 1) * P],
                                rhs=xt_sb[:, k, t * 512:(t + 1) * 512],
                                start=(k == 0),
                                stop=(k == KM - 1),
                            )
                        nc.scalar.activation(
                            hT[:, h_abs, t * 512:(t + 1) * 512],
                            ps[:],
                            GELU,
                            bias=b1_sb[:, h_abs:h_abs + 1],
                        )

        # ---- phase B: out = hT.T @ W2, 512 tokens at a time ----
        with tc.tile_pool(name="psB" + sfx, bufs=1, space="PSUM") as psB_pool:
            psB = psB_pool.tile([P, 8 * 512], F32)   # all 8 banks
            for half in range(2):
                for hk in range(NH):
                    for t in range(4):
                        tt = half * 4 + t
                        for oc in range(2):
                            r = t * 2 + oc
                            nc.tensor.matmul(
                                psB[:, r * 512:(r + 1) * 512],
                                lhsT=hT[:, hk, tt * P:(tt + 1) * P],
                                rhs=w2_sb[:, hk, oc * 512:(oc + 1) * 512],
                                start=(hk == 0),
                                stop=(hk == NH - 1),
                            )
                for t in range(4):
                    tt = half * 4 + t
                    for oc in range(2):
                        r = t * 2 + oc
                        ot = out_pool.tile([P, 512], F32, tag="ot")
                        nc.vector.tensor_copy(ot[:], psB[:, r * 512:(r + 1) * 512])
                        nc.sync.dma_start(
                            out=out[tt * P:(tt + 1) * P, oc * 512:(oc + 1) * 512],
                            in_=ot[:],
                        )


def _run_device(in_maps):
    global LAST_RESULTS, _NC_CACHE
    from concourse.bass_utils import run_bass_kernel_spmd

    if _NC_CACHE is None:
        _NC_CACHE = _build_nc()
    res = run_bass_kernel_spmd(_NC_CACHE, in_maps, core_ids=list(range(NCORES)))
    LAST_RESULTS = res
    return [r["out"] for r in res.results]


def _run_numpy(in_maps):
    """Host fallback mirroring the device math (for routing validation)."""
    outs = []
    for m in in_maps:
        x = m["xt"].astype(np.float32).T           # [tok, M]
        w1 = m["w1"].astype(np.float32)
        w2 = m["w2"].astype(np.float32)
        b1 = m["b1"].T.reshape(-1)                 # [H]
        h = x @ w1 + b1
        g = 0.5 * h * (1.0 + np.tanh(0.7978845608028654 * (h + 0.044715 * h ** 3)))
        outs.append((g @ w2).astype(np.float32))
    return outs


# --------------------------------------------------------------------------
# Entry point
# --------------------------------------------------------------------------

def kernel(hidden_states, gate_w, fc1_w, fc1_b, fc2_w, fc2_b):
    hidden_states = np.asarray(hidden_states)
    features = hidden_states.reshape(S, M)
    gate_w = np.asarray(gate_w)
    fc1_w, fc1_b = np.asarray(fc1_w), np.asarray(fc1_b)
    fc2_w, fc2_b = np.asarray(fc2_w), np.asarray(fc2_b)

    idx, keep, gate_val, l_aux, toks_per_e = _route(features, gate_w)

    # ---- shard: gather+transpose tokens per core, cast weights to bf16 ----
    in_maps = []
    for core in range(NCORES):
        e, half = core // 2, core % 2
        toks = toks_per_e[e]
        lo, hi = half * TOK, min((half + 1) * TOK, len(toks))
        xt = np.zeros((M, TOK), BF16)
        if hi > lo:
            xt[:, :hi - lo] = features[toks[lo:hi]].astype(BF16).T
        in_maps.append({
            "xt": xt,
            "w1": fc1_w[e].astype(BF16),
            "b1": np.ascontiguousarray(fc1_b[e].reshape(H // P, P).T.astype(np.float32)),
            "w2": fc2_w[e].astype(BF16),
        })

    if os.environ.get("MOE_NUMPY_MLP"):
        core_outs = _run_numpy(in_maps)
    else:
        core_outs = _run_device(in_maps)

    # ---- combine: scatter expert outputs back to token order ----
    out_full = np.zeros((S, O), np.float32)
    for e in range(E):
        toks = toks_per_e[e]
        n = len(toks)
        eo = np.concatenate([core_outs[2 * e], core_outs[2 * e + 1]], axis=0)[:n]
        eo = eo + fc2_b[e][None, :].astype(np.float32)
        out_full[toks] = gate_val[toks, None] * eo

    return out_full.reshape(B, T, O), l_aux
